# revision 3
# baseline (speedup 1.0000x reference)
"""Kernelized (linear) attention on 8 TRN2 NeuronCores — v21 (final): v17 + DoubleRowSwInterleave main-loop matmuls.

Single-collective design (vs v11's four):
  - Full Wk/Wv/Wo staged per core in device DRAM (no weight AllGather);
    main loop is T-data-parallel over the 8 cores as before.
  - ONE AllGather carries: per-core kv/ks stat partials ([128,64]+[128,64])
    plus that core's relu'd q-projection slice ([128,8]) — rank placement
    in the AG output gives per-core column offsets for free in the
    uniform SPMD program.
  - Every core then sums the 8 stat chunks locally, assembles the full
    q_k, and computes ITS 256-column slice of the output (16 FD=256
    matmuls against its Wo column slice); the host concatenates the 8
    slices — no ReduceScatter / output AllReduce.
  - AG payload is bf16 to halve wire/readback cost; kk intermediate is
    bf16 to cut SBUF traffic.

Layout per core c (hardcoded D=2048, H=16, T=4096, B=4, 8 cores):
  xk/xv: [D, R=2048] fp8, columns r = b*512 + t_local, t in [c*512,(c+1)*512)
  wkv:   [NT*256, D] fp8: rows [ot*256, ot*256+128) = Wk block ot,
         [+128, +256) = Wv block ot, in wblocks layout
         [p = d%128, (d//128)*128 + o_in], host-scaled by WSC
  wq:    [2, 128, D] fp8 wblocks layout for own ot = 2c+otl
  wo:    [NT, 128, D] fp8 = Wo.T[ot block rows, :] for ALL ot, x WSC
  qt:    [128, NT*B] fp8, [p = d%128, (d//128)*B + b]
  AG payload agin [128, 136] f32: cols 0:64 kv partials (col ot*B+b),
         64:128 ks partials, 128:136 own relu(q-proj) (ot-major, b-minor)
"""

import numpy as np
import ml_dtypes

from concourse import bass, bacc, mybir, tile
from concourse.bass_utils import run_bass_kernel_spmd

BF16 = ml_dtypes.bfloat16
F8H = ml_dtypes.float8_e4m3

D, H, T, B = 2048, 16, 4096, 4
HD = D // H           # 128
NCORES = 8
TLOC = T // NCORES    # 512 history rows per core
R = TLOC * B          # 2048 projection rows per core
NT = D // 128         # 16 tiles along d (contraction) and o (output)
NTT = NT // 2         # 8 DoubleRow contraction pairs
OTC = NT // NCORES    # 2 output tiles (heads) owned per core
EPS = 1e-6
F32 = mybir.dt.float32
BF = mybir.dt.bfloat16
F8 = mybir.dt.float8e4
WSC = 64.0    # host pre-scale on Wk/Wv/Wq/Wo so fp8 stays in normal range
OSC = 8192.0  # pre-scale on opre (tiny values) before fp8 cast
AF = mybir.ActivationFunctionType
OP = mybir.AluOpType
AGC = 136     # AG payload columns: 64 kv + 64 ks + 8 qk


def build_nc(K=1):
    nc = bacc.Bacc("TRN2", target_bir_lowering=False, debug=False,
                   enable_asserts=False, num_devices=NCORES)

    def din(name, shape, dt):
        return nc.dram_tensor(name, list(shape), dt, kind="ExternalInput").ap()

    xk_d = din("xk", (NTT * 128, 2 * R), F8)
    xv_d = din("xv", (NTT * 128, 2 * R), F8)
    wkv_d = din("wkv", (NT * 128, 2 * D), F8)
    wq_d = din("wq", (OTC, 128, D), F8)
    wo_d = din("wo", (NT, 128, D // NCORES), F8)
    qt_d = din("qt", (128, NT * B), F8)
    bk_d = din("bk", (128, NT), F32)
    bv64_d = din("bv64", (128, NT * B), F32)
    bq_d = din("bq", (128, OTC), F32)
    bo_d = din("bo", (B, D // NCORES), F32)
    al_d = din("al", (1, NT * B), F32)
    onc_d = din("onc", (128, 1), F32)
    onr_d = din("onr", (1, 128), F32)

    out_d = nc.dram_tensor("out", [B, D // NCORES], F32,
                           kind="ExternalOutput").ap()

    with tile.TileContext(nc) as tc:
        import contextlib
        with contextlib.ExitStack() as ctx:
            p_xk = ctx.enter_context(tc.tile_pool(name="xk", bufs=NT))
            p_xv = ctx.enter_context(tc.tile_pool(name="xv", bufs=NT))
            p_w = ctx.enter_context(tc.tile_pool(name="w", bufs=4))
            p_wo = ctx.enter_context(tc.tile_pool(name="wo", bufs=4))
            p_ep = ctx.enter_context(tc.tile_pool(name="ep", bufs=3))
            p_pr = ctx.enter_context(tc.tile_pool(name="pr", bufs=2))
            p_c1 = ctx.enter_context(tc.tile_pool(name="c1", bufs=1))
            p_c2 = ctx.enter_context(tc.tile_pool(name="c2", bufs=2))
            p_st = ctx.enter_context(tc.tile_pool(name="st", bufs=2))
            p_dr = ctx.enter_context(tc.tile_pool(name="dr", bufs=2,
                                                  space="DRAM"))
            p_mk = ctx.enter_context(
                tc.tile_pool(name="mmk", bufs=3, space="PSUM"))
            p_mv = ctx.enter_context(
                tc.tile_pool(name="mmv", bufs=3, space="PSUM"))
            p_fin = ctx.enter_context(
                tc.tile_pool(name="fin", bufs=1, space="PSUM"))
            p_op = ctx.enter_context(
                tc.tile_pool(name="opp", bufs=1, space="PSUM"))

            DRm = mybir.MatmulPerfMode.DoubleRowSwInterleave

            for _it in range(K):
                # ---- small resident loads ------------------------------
                qt_s = p_c1.tile([128, NT * B], F8, tag="qt")
                nc.sync.dma_start(out=qt_s[:], in_=qt_d[:, :])
                bk_s = p_c1.tile([128, NT], F32, tag="bk")
                nc.sync.dma_start(out=bk_s[:], in_=bk_d[:, :])
                bv64_s = p_c1.tile([128, NT * B], F32, tag="bv64")
                nc.sync.dma_start(out=bv64_s[:], in_=bv64_d[:, :])
                bq_s = p_c1.tile([128, OTC], F32, tag="bq")
                nc.sync.dma_start(out=bq_s[:], in_=bq_d[:, :])
                bo_s = p_c1.tile([B, D // NCORES], F32, tag="bo")
                nc.sync.dma_start(out=bo_s[:], in_=bo_d[:, :])
                al_s = p_c1.tile([1, NT * B], F32, tag="al")
                nc.sync.dma_start(out=al_s[:], in_=al_d[:, :])
                onc_s = p_c1.tile([128, 1], F32, tag="onc")
                nc.sync.dma_start(out=onc_s[:], in_=onc_d[:, :])
                onr_s = p_c1.tile([1, 128], F32, tag="onr")
                nc.sync.dma_start(out=onr_s[:], in_=onr_d[:, :])
                wq_s = []
                for ol in range(OTC):
                    w = p_c1.tile([128, D], F8, tag=f"wq{ol}")
                    nc.sync.dma_start(out=w[:], in_=wq_d[ol])
                    wq_s.append(w)


                # ---- AG staging buffer (stats accumulate into it) ------
                arst = p_c2.tile([128, AGC], F32, tag="arst")

                # ---- q projection for own 2 heads ----------------------
                qp = p_fin.tile([128, OTC * B], F32, tag="tn",
                                name=f"qp{_it}")
                for ol in range(OTC):
                    for t in range(NT):
                        nc.tensor.matmul(
                            qp[:, ol * B:(ol + 1) * B],
                            wq_s[ol][:, t * 128:(t + 1) * 128],
                            qt_s[:, t * B:(t + 1) * B],
                            start=(t == 0), stop=(t == NT - 1))
                for ol in range(OTC):
                    nc.scalar.activation(
                        arst[:, 128 + ol * B:128 + (ol + 1) * B],
                        qp[:, ol * B:(ol + 1) * B], AF.Relu,
                        bias=bq_s[:, ol:ol + 1], scale=1.0 / WSC)

                # ---- resident x loads ----------------------------------
                xk_t, xv_t = [], []
                for tt in range(NTT):
                    a = p_xk.tile([128, 2, R], F8, tag="xk")
                    nc.sync.dma_start(out=a[:],
                                      in_=xk_d[tt * 128:(tt + 1) * 128, :])
                    xk_t.append(a)
                    b_ = p_xv.tile([128, 2, R], F8, tag="xv")
                    nc.sync.dma_start(out=b_[:],
                                      in_=xv_d[tt * 128:(tt + 1) * 128, :])
                    xv_t.append(b_)

                # ---- main loop: K/V projections + fused stats ----------
                for ot in range(NT):
                    wkv_s = p_w.tile([128, 2, NTT, 2, 128], F8, tag="wkv")
                    nc.scalar.dma_start(
                        out=wkv_s[:],
                        in_=wkv_d[ot * 128:(ot + 1) * 128, :])
                    wk_s = wkv_s[:, 0]
                    wv_s = wkv_s[:, 1]
                    for b in range(B):
                        c0 = b * 512
                        ck = ot * B + b
                        kp = p_mk.tile([128, 512], F32, tag="mmk")
                        for tt in range(NTT):
                            nc.tensor.matmul(
                                kp[:], wk_s[:, tt],
                                xk_t[tt][:, :, c0:c0 + 512],
                                start=(tt == 0), stop=(tt == NTT - 1),
                                perf_mode=DRm)
                        vp = p_mv.tile([128, 512], F32, tag="mmv")
                        for tt in range(NTT):
                            nc.tensor.matmul(
                                vp[:], wv_s[:, tt],
                                xv_t[tt][:, :, c0:c0 + 512],
                                start=(tt == 0), stop=(tt == NTT - 1),
                                perf_mode=DRm)
                        kk = p_ep.tile([128, 512], BF, tag="kk")
                        nc.scalar.activation(
                            kk[:], kp[:], AF.Relu,
                            bias=bk_s[:, ot:ot + 1], scale=1.0 / WSC,
                            accum_out=arst[:, 64 + ck:64 + ck + 1])
                        pr = p_pr.tile([128, 512], BF, tag="pr")
                        nc.vector.scalar_tensor_tensor(
                            pr[:], kk[:], EPS, vp[:], OP.add, OP.mult,
                            accum_out=arst[:, ck:ck + 1])

                # ---- single AllGather: stats + own q_k -----------------
                ag_in = p_dr.tile([128, AGC], BF, tag="agin")
                nc.gpsimd.dma_start(out=ag_in[:], in_=arst[:])
                ag_out = p_dr.tile([NCORES * 128, AGC], BF, tag="agout",
                                   addr_space="Shared")
                nc.gpsimd.collective_compute(
                    "AllGather", OP.bypass,
                    replica_groups=[list(range(NCORES))],
                    ins=[ag_in.opt()], outs=[ag_out.opt()])

                # ---- local reduce of the 8 chunks ----------------------
                chunks = []
                for r in range(NCORES):
                    ch = p_st.tile([128, AGC], BF, tag=f"ch{r % 2}",
                                   name=f"ch{r}_{_it}")
                    nc.sync.dma_start(out=ch[:],
                                      in_=ag_out[r * 128:(r + 1) * 128, :])
                    chunks.append(ch)
                g = p_c2.tile([128, 128], F32, tag="g")
                nc.vector.tensor_tensor(g[:], chunks[0][:, 0:128],
                                        chunks[1][:, 0:128], OP.add)
                for r in range(2, NCORES):
                    nc.vector.tensor_tensor(g[:], g[:],
                                            chunks[r][:, 0:128], OP.add)
                qk = p_c2.tile([128, NT * B], F32, tag="qk")
                for r in range(NCORES):
                    nc.vector.tensor_copy(qk[:, r * 8:(r + 1) * 8],
                                          chunks[r][:, 128:136])

                # ---- combine stats (all 16 heads) ----------------------
                hs = p_fin.tile([1, NT * B], F32, tag="tn",
                                name=f"hs{_it}")
                nc.tensor.matmul(hs[:], onc_s[:], g[:, 64:128],
                                 start=True, stop=True)
                den = p_c2.tile([1, NT * B], F32, tag="den")
                nc.vector.tensor_scalar(den[:], hs[:], EPS * T * HD + EPS,
                                        None, OP.add)
                rden = p_c2.tile([1, NT * B], F32, tag="rden")
                nc.vector.reciprocal(rden[:], den[:])
                rr = p_c2.tile([1, NT * B], F32, tag="rr")
                nc.vector.tensor_tensor(rr[:], rden[:], al_s[:], OP.mult)
                bcr_ps = p_fin.tile([128, NT * B], F32, tag="tn",
                                    name=f"bcr{_it}")
                nc.tensor.matmul(bcr_ps[:], onr_s[:], rr[:], start=True,
                                 stop=True)
                kvb = p_st.tile([128, NT * B], F32, tag="kvb")
                nc.vector.scalar_tensor_tensor(
                    kvb[:], g[:, 64:128], T * EPS, bv64_s[:], OP.add,
                    OP.mult)
                kvc = p_st.tile([128, NT * B], F32, tag="kvc")
                nc.vector.scalar_tensor_tensor(
                    kvc[:], g[:, 0:64], 1.0 / WSC, kvb[:], OP.mult, OP.add)
                kvr = p_c2.tile([128, NT * B], F32, tag="kvr")
                nc.vector.tensor_tensor(kvr[:], kvc[:], bcr_ps[:], OP.mult)
                opre = p_c2.tile([128, NT * B], F8, tag="opre")
                nc.vector.scalar_tensor_tensor(
                    opre[:], qk[:], EPS, kvr[:], OP.add, OP.mult)

                # ---- Wo apply: own 256-column slice only ---------------
                OC = D // NCORES
                op_ps = p_op.tile([B, OC], F32, tag="opp",
                                  name=f"op_{_it}")
                for ol in range(NT):
                    w = p_wo.tile([128, OC], F8, tag="wo")
                    nc.scalar.dma_start(out=w[:], in_=wo_d[ol])
                    nc.tensor.matmul(
                        op_ps[:], opre[:, ol * B:(ol + 1) * B], w[:],
                        start=(ol == 0), stop=(ol == NT - 1))
                opart = p_st.tile([B, OC], F32, tag="opart")
                nc.vector.scalar_tensor_tensor(
                    opart[:], op_ps[:], 1.0 / (OSC * WSC),
                    bo_s[:], OP.mult, OP.add)
                nc.sync.dma_start(out=out_d[:, :], in_=opart[:])

    nc.finalize()
    from concourse import bass_interp
    nc.m = bass_interp.get_hw_module(nc.m)
    return nc


def prep_inputs(q, k_history, v_history, Wq, bq, Wk, bk, Wv, bv, Wo, bo,
                alpha):
    """Host-side sharding + layout transforms. Returns in_maps for 8 cores."""
    f32 = np.float32

    def wblocks(W):  # [o,d] -> [ot, p(d%128), (d//128)*128 + o_in] f32
        a = W.astype(f32).reshape(NT, 128, NT, 128)       # (ot, o_in, t, p)
        return np.ascontiguousarray(a.transpose(0, 3, 2, 1)) \
                 .reshape(NT, 128, D)

    wkb = wblocks(Wk)
    wvb = wblocks(Wv)
    wqb = wblocks(Wq)
    wob = np.ascontiguousarray(
        Wo.astype(f32).T.reshape(NT, 128, D))               # [ot, p(o_in), o']
    qt = np.ascontiguousarray(
        q.astype(f32).T.reshape(NT, 128, B).transpose(1, 0, 2)
    ).reshape(128, NT * B).astype(F8H)                      # [p, t*4+b]
    bk_t = np.ascontiguousarray(bk.astype(f32).reshape(NT, 128).T)
    bv_t = bv.astype(f32).reshape(NT, 128).T                # [128, NT]
    bv64 = np.ascontiguousarray(np.repeat(bv_t, B, axis=1))  # [128, NT*B]
    bq_t = np.ascontiguousarray(bq.astype(f32).reshape(NT, 128).T)
    bo_r = np.ascontiguousarray(np.tile(bo.astype(f32)[None, :], (B, 1)))
    onc = np.ones((128, 1), f32)
    onr = np.ones((1, 128), f32)
    alpha = np.asarray(alpha, f32)
    al = np.ascontiguousarray(np.repeat(alpha, B)[None, :] * OSC)

    # per-ot interleave [p][kv][tt][s][o] -> [NT*128, 2*D], scaled for fp8
    wkv = np.stack([wkb, wvb], axis=2)          # [NT, 128, 2, D]
    wkv = np.ascontiguousarray(wkv * WSC).astype(F8H).reshape(NT * 128, 2 * D)
    wo_full = np.ascontiguousarray(wob * WSC).astype(F8H)

    shared = dict(qt=qt, bk=bk_t, bv64=bv64, al=al, onc=onc,
                  onr=onr, wkv=wkv)

    kb = np.asarray(k_history, f32).astype(F8H)             # [T, B, D]
    vb = np.asarray(v_history, f32).astype(F8H)

    in_maps = []
    for c in range(NCORES):
        def xl(h):  # [TLOC,B,D] -> [NTT*128, 2*R]: [tt][p][s][b][t]
            a = h[c * TLOC:(c + 1) * TLOC].transpose(2, 1, 0)  # [D,B,TLOC]
            a = a.reshape(NTT, 2, 128, B * TLOC)               # [tt,s,p,r]
            return np.ascontiguousarray(
                a.transpose(0, 2, 1, 3)).reshape(NTT * 128, 2 * R)
        xk = xl(kb)
        xv = xl(vb)
        oc = D // NCORES
        in_maps.append(dict(
            xk=xk, xv=xv,
            wq=(np.ascontiguousarray(wqb[OTC * c:OTC * (c + 1)])
                * WSC).astype(F8H),
            bq=np.ascontiguousarray(bq_t[:, OTC * c:OTC * (c + 1)]),
            wo=np.ascontiguousarray(wo_full[:, :, c * oc:(c + 1) * oc]),
            bo=np.ascontiguousarray(bo_r[:, c * oc:(c + 1) * oc]),
            **shared))
    return in_maps


_CACHE = {}


def kernel(**inputs):
    if "nc" not in _CACHE:
        _CACHE["nc"] = build_nc(K=1)
    nc = _CACHE["nc"]
    in_maps = prep_inputs(**{k: np.asarray(v) for k, v in inputs.items()})
    res = run_bass_kernel_spmd(nc, in_maps, core_ids=list(range(NCORES)))
    return np.concatenate(
        [np.asarray(res.results[c]["out"], dtype=np.float32)
         for c in range(NCORES)], axis=1)
